# revision 49
# baseline (speedup 1.0000x reference)
"""Trainium2 Bass kernel for nn_DistSelfAttention (distributional self-attention).

Sharding (Megatron-style, per the sharding hint): 8 cores = 2 batches x 4
head groups. Core c handles batch b=c//4, heads 4g..4g+3 where g=c%4 (a
contiguous 256-wide slice of the 1024 model dim). Q/K/V projection weights
are column-split, the output projection is row-split with an on-device
ReduceScatter over each batch's 4-core group; each core emits a 256-row
s-quarter of out_loc / out_scale which the host reassembles.

Math notes (vs the reference):
  - q_scale / k_scale projections are dead code in the reference (score
    variance is a constant) and are skipped.
  - Scores are computed transposed (score^T[t,s]) so the exp'd tiles stream
    straight into the attention-value matmuls as the moving operand; the
    softmax denominator r_top comes from a packed ones column.
  - Softmax normalization is deferred: y = (expU @ V) / rowsum(expU). Both
    softmaxes are normalized by r_top (r_rest/r_top - 1 ~ 1e-4), the rest^2
    variance correction (~2.5e-4 relative) and the (sqrt(u)+eps)^2 expansion
    of vvar (~4.5e-5) are dropped; all are far below the float32r matmul
    noise (~2e-4).

Hardware constraints baked in (discovered against the walrus verifier / HW):
  - f32r matmuls need full 128-column tiling: stationary column counts must
    round to 128 with useful columns in 0..63 or a full [x|zeros|y] layout;
    no column tile_position; even moving free size.
  - ACT cannot read f32r (bitcast to f32), can write it. DVE ops need all
    SBUF operands at one base partition. The custom reciprocal op and
    gpsimd partition_broadcast only work from partition 0. DMA cannot touch
    PSUM; only sync/scalar/gpsimd issue DMAs.
"""

import sys

sys.path.insert(0, "/opt/trn_rl_repo")

import math

import numpy as np

import concourse.tile as tile
from concourse import bacc, mybir
from concourse.bass_utils import run_bass_kernel_spmd

F32 = mybir.dt.float32
F32R = mybir.dt.float32r
AF = mybir.ActivationFunctionType
ALU = mybir.AluOpType

B, S, D, H = 2, 1024, 1024, 16
HD = D // H  # 64
NCORES = 8
G = 4  # head groups (cores per batch)
HPG = H // G  # heads per core = 4
DL = D // G  # local model-dim slice = 256
EPS = 1e-6
NT = S // 128  # 8 sequence tiles

# Single-core timing build: num_devices=1 and the ReduceScatter replaced by a
# local DMA so TimelineSim (cost model) can estimate per-core exec time.
TIMING_SINGLE = False

_CACHE = {}


def _build(tau: float):
    inv8tau = 1.0 / (math.sqrt(HD) * tau)
    s_var = (0.1 + EPS) / HD + EPS
    l_var = s_var / (tau * tau) + EPS
    c_adj = 1.0 / math.sqrt(1.0 + (math.pi / 8.0) * l_var)
    rest_scale = c_adj * inv8tau

    nc = bacc.Bacc(
        "TRN2", target_bir_lowering=False, debug=False,
        num_devices=(1 if TIMING_SINGLE else NCORES),
    )

    # activation() float biases need pre-registered const APs (only 0.0/1.0
    # exist by default) -- mirror Bass.__init__'s registration for EPS
    _eps_t = nc.alloc_sbuf_tensor(f"const-float32-{EPS}", [128, 1], F32)
    nc.gpsimd.memset(_eps_t.ap(), EPS)
    nc.const_aps.aps[(F32, EPS)] = _eps_t.ap()
    nc.all_engine_barrier()

    def din(name, shape, dt=F32):
        return nc.dram_tensor(name, shape, dt, kind="ExternalInput").ap()

    # transposed activations: [din, s]
    xq = din("xq", [D, S], F32R)
    xk = din("xk", [D, S], F32R)
    xv = din("xv", [D, S], F32R)          # v_loc[b].T
    xvs2 = din("xvs2", [D, S], F32R)      # (v_scale[b].T)**2
    # weights
    wq = din("wq", [D, DL], F32R)         # Wq[dsl].T
    wk = din("wk", [D, DL], F32R)
    wv = din("wv", [D, DL], F32R)
    wo = din("wo", [DL, D], F32R)         # Wo.T[dsl]
    # biases / constants
    bq = din("bq", [DL, 1])
    bk = din("bk", [DL, 1])
    bvb = din("bvb", [128, DL])           # bv slice broadcast over partitions
    bob = din("bob", [128, D])            # bo broadcast over partitions
    cvec = din("cvec", [128, 1])          # attn_const column for vvar colsums
    onew = din("onew", [128, 2048], F32R) # [1,0*63] blocks: vmuP ones columns

    out_loc = nc.dram_tensor("out_loc", [DL, D], F32, kind="ExternalOutput").ap()
    out_scale = nc.dram_tensor("out_scale", [DL, D], F32, kind="ExternalOutput").ap()

    cc_in = nc.dram_tensor("cc_in", [2 * S, D], F32)
    cc_out = nc.dram_tensor("cc_out", [2 * DL, D], F32)

    with tile.TileContext(nc) as tc:
        with tc.tile_pool(name="persist", bufs=1) as pp:
            qmuT = pp.tile([128, 2, S], F32R, tag="qmuT")
            kmuT = pp.tile([128, 2, S], F32R, tag="kmuT")
            # per-t-tile V stationaries so attention can start while the V
            # projection is still streaming:
            #   VB_t[tt]   = per head [vvar | vmu]         (rest stream)
            #   vmuP_t[tt] = per head [ones | zeros | vmu] (top stream, r_top)
            VB_t = [
                pp.tile([128, 2 * HD * HPG], F32R, tag=f"VB{t}", name=f"VB{t}")
                for t in range(NT)
            ]
            vmuP_t = [
                pp.tile([128, 2 * HD * HPG], F32R, tag=f"vP{t}", name=f"vP{t}")
                for t in range(NT)
            ]
            ymu_sb = pp.tile([128, 2, S], F32R, tag="ymu")
            yvar_sb = pp.tile([128, 2, S], F32, tag="yvar")
            ysq_sb = pp.tile([128, 2, S], F32R, tag="ysq")
            colsum_sb = pp.tile([128, HPG, 1], F32, tag="colsum")
            cv_sb = pp.tile([128, 1], F32, tag="cvec")
            wo_sb = pp.tile([128, 2, D], F32R, tag="wo")
            wo2_sb = pp.tile([128, 2, D], F32R, tag="wo2")

            wq_sb = pp.tile([128, 8, DL], F32R, tag="wq")
            wk_sb = pp.tile([128, 8, DL], F32R, tag="wk")
            wv_sb = pp.tile([128, 8, DL], F32R, tag="wv")
            wv2_sb = pp.tile([128, 8, DL], F32R, tag="wv2")
            bq_sb = pp.tile([128, 2, 1], F32, tag="bq")
            bk_sb = pp.tile([128, 2, 1], F32, tag="bk")
            bvb_sb = pp.tile([128, DL], F32, tag="bvb")

            if True:
                # critical-path weights first on the gpsimd queue
                for w_sb, w_d in ((wq_sb, wq), (wk_sb, wk), (wv_sb, wv)):
                    nc.gpsimd.dma_start(
                        out=w_sb[:], in_=w_d.rearrange("(t p) c -> p t c", p=128)
                    )
                nc.gpsimd.dma_start(
                    out=bq_sb[:], in_=bq.rearrange("(t p) c -> p t c", p=128)
                )
                nc.gpsimd.dma_start(
                    out=bk_sb[:], in_=bk.rearrange("(t p) c -> p t c", p=128)
                )
                nc.gpsimd.dma_start(out=bvb_sb[:], in_=bvb)
                nc.vector.tensor_mul(
                    wv2_sb[:], wv_sb[:].bitcast(F32), wv_sb[:].bitcast(F32)
                )
                # lower-priority constants behind the projection weights
                nc.gpsimd.dma_start(out=cv_sb[:], in_=cvec)
                for t in range(NT):
                    nc.gpsimd.dma_start(
                        out=vmuP_t[t][:].rearrange("p (h x) -> p h x", h=HPG)[
                            :, :, 0:HD
                        ],
                        in_=onew[:, 256 * t : 256 * t + 256],
                    )
                nc.gpsimd.dma_start(
                    out=wo_sb[:], in_=wo.rearrange("(t p) c -> p t c", p=128)
                )
                nc.vector.tensor_mul(
                    wo2_sb[:], wo_sb[:].bitcast(F32), wo_sb[:].bitcast(F32)
                )

            # --- Q/K transposed projections: out[dl_tile, s] ---
            with (
                tc.tile_pool(name="xqkp", bufs=2) as xqkp,
                tc.tile_pool(name="pjps", bufs=2, space="PSUM") as pjps,
            ):
                for w_sb, x_dram, b_sb, o_sb in (
                    (wq_sb, xq, bq_sb, qmuT),
                    (wk_sb, xk, bk_sb, kmuT),
                ):
                    for j in range(2):
                        cs = slice(512 * j, 512 * j + 512)
                        ps0 = pjps.tile([128, 512], F32, tag="pj")
                        ps1 = pjps.tile([128, 512], F32, tag="pj")
                        for kh in range(2):
                            xt = xqkp.tile([128, 4, 512], F32R, tag="xqk")
                            nc.sync.dma_start(
                                out=xt[:],
                                in_=x_dram.rearrange("(t p) s -> p t s", p=128)[
                                    :, 4 * kh : 4 * kh + 4, cs
                                ],
                            )
                            for kq in range(4):
                                kt = 4 * kh + kq
                                for mt, ps in ((0, ps0), (1, ps1)):
                                    nc.tensor.matmul(
                                        ps[:],
                                        lhsT=w_sb[:, kt, 128 * mt : 128 * mt + 128],
                                        rhs=xt[:, kq, :],
                                        start=(kt == 0),
                                        stop=(kt == 7),
                                    )
                        for mt, ps in ((0, ps0), (1, ps1)):
                            nc.scalar.activation(
                                o_sb[:, mt, cs], ps[:], AF.Identity,
                                bias=b_sb[:, mt, :],
                            )

            # ===== V projections + attention (shared PSUM slots) =====
            with (
                tc.tile_pool(name="xin", bufs=2) as xin,
                tc.tile_pool(name="expp", bufs=2) as expp,
                tc.tile_pool(name="normp", bufs=1) as normp,
                tc.tile_pool(name="scps", bufs=2, space="PSUM") as scps,
                tc.tile_pool(name="accps", bufs=2, space="PSUM") as accps,
            ):
                # --- V mean + var projections: out[t_tile, dl] ---
                for mt in range(NT):
                    ms = slice(128 * mt, 128 * mt + 128)
                    xvt = xin.tile([128, 8, 128], F32R, tag="xv")
                    nc.sync.dma_start(
                        out=xvt[:],
                        in_=xv.rearrange("(t p) s -> p t s", p=128)[:, :, ms],
                    )
                    xst = xin.tile([128, 8, 128], F32R, tag="xvs")
                    nc.sync.dma_start(
                        out=xst[:],
                        in_=xvs2.rearrange("(t p) s -> p t s", p=128)[:, :, ms],
                    )
                    psv = scps.tile([128, DL], F32, tag="sc")
                    psu = scps.tile([128, DL], F32, tag="sc")
                    for kt in range(8):
                        nc.tensor.matmul(
                            psv[:], lhsT=xvt[:, kt, :], rhs=wv_sb[:, kt, :],
                            start=(kt == 0), stop=(kt == 7),
                        )
                        nc.tensor.matmul(
                            psu[:], lhsT=xst[:, kt, :], rhs=wv2_sb[:, kt, :],
                            start=(kt == 0), stop=(kt == 7),
                        )
                    # vmu = psv + bv  -> VB (cols 64-127 per head) and vmuP
                    vb_mu = VB_t[mt][:].rearrange("p (h x) -> p h x", h=HPG)[
                        :, :, HD : 2 * HD
                    ]
                    nc.vector.tensor_add(
                        vb_mu,
                        psv[:].rearrange("p (h d) -> p h d", h=HPG),
                        bvb_sb[:].rearrange("p (h d) -> p h d", h=HPG),
                    )
                    vp_mu = vmuP_t[mt][:].rearrange("p (h x) -> p h x", h=HPG)[
                        :, :, HD : 2 * HD
                    ]
                    nc.scalar.copy(vp_mu, vb_mu.bitcast(F32))
                    # vvar = (sqrt(u)+eps)^2 ~= u  (correction ~4.5e-5 rel)
                    vb_var = VB_t[mt][:].rearrange("p (h x) -> p h x", h=HPG)[
                        :, :, 0:HD
                    ]
                    nc.vector.tensor_copy(
                        vb_var, psu[:].rearrange("p (h d) -> p h d", h=HPG)
                    )

                # --- per-head column-sums of vvar (const part of y_var) ---
                # plain fp32 matmuls (N=1 is illegal for f32r)
                for h in range(HPG):
                    csp = accps.tile([128, 1], F32, tag="acc")
                    for tt in range(NT):
                        nc.tensor.matmul(
                            csp[0:HD, :],
                            lhsT=VB_t[tt][:, 2 * HD * h : 2 * HD * h + HD].bitcast(F32),
                            rhs=cv_sb[:],
                            start=(tt == 0),
                            stop=(tt == NT - 1),
                        )
                    nc.vector.tensor_copy(colsum_sb[0:HD, h, :], csp[0:HD, :])

                # --- attention, head by head ---
                for h in range(HPG):
                    doff = HD * h
                    mt_h = doff // 128
                    off = doff % 128
                    hs = slice(2 * HD * h, 2 * HD * (h + 1))
                    y1 = accps.tile([128, S], F32, tag="acc")  # r_top | top@vmu
                    y2 = accps.tile([128, S], F32, tag="acc")  # rest@[vvar|vmu]
                    for tt in range(NT):
                        topU = expp.tile([128, S], F32R, tag="topU")
                        restU = expp.tile([128, S], F32R, tag="restU")
                        sc = scps.tile([128, S], F32, tag="sc")
                        for j in range(2):
                            sl = slice(512 * j, 512 * j + 512)
                            nc.tensor.matmul(
                                sc[:, sl],
                                lhsT=kmuT[off : off + HD, mt_h, 128 * tt : 128 * tt + 128],
                                rhs=qmuT[off : off + HD, mt_h, sl],
                                start=True,
                                stop=True,
                            )
                        nc.scalar.activation(topU[:], sc[:], AF.Exp, scale=inv8tau)
                        nc.scalar.activation(restU[:], sc[:], AF.Exp, scale=rest_scale)
                        for j in range(2):
                            sl = slice(512 * j, 512 * j + 512)
                            st = tt == 0
                            sp = tt == NT - 1
                            nc.tensor.matmul(
                                y1[:, sl], lhsT=vmuP_t[tt][:, hs],
                                rhs=topU[:, sl], start=st, stop=sp,
                            )
                            nc.tensor.matmul(
                                y2[:, sl], lhsT=VB_t[tt][:, hs],
                                rhs=restU[:, sl], start=st, stop=sp,
                            )

                    # --- normalization (both softmaxes by 1/r_top) ---
                    rw = normp.tile([1, 2, S], F32, tag="rw")
                    nc.vector.reciprocal_approx_fast(
                        out=rw[0:1, 0, :], in_=y1[0:1, :]
                    )
                    bTop = normp.tile([128, S], F32, tag="bTop")
                    nc.gpsimd.partition_broadcast(bTop[:], rw[0:1, 0, :])

                    # mean chain @ partitions 64-127
                    tmp1 = normp.tile([128, S], F32, tag="tmp1")
                    nc.vector.tensor_mul(tmp1[HD:128, :], y1[HD:128, :], bTop[HD:128, :])
                    t23a = normp.tile([128, S], F32, tag="t23a")
                    nc.vector.tensor_mul(t23a[HD:128, :], y2[HD:128, :], bTop[HD:128, :])
                    ymu_t = normp.tile([128, S], F32R, tag="ymu_t")
                    nc.vector.tensor_add(
                        ymu_t[HD:128, :], tmp1[HD:128, :], t23a[HD:128, :]
                    )
                    nc.gpsimd.dma_start(
                        out=ymu_sb[off : off + HD, mt_h, :], in_=ymu_t[HD:128, :]
                    )

                    # variance chain @ partitions 0-63:
                    # y_var = l_var*(rest@vvar)/r_top + attn_const*colsum
                    t23b = normp.tile([HD, S], F32, tag="t23b")
                    nc.vector.tensor_mul(t23b[:], y2[0:HD, :], bTop[0:HD, :])
                    yvar_t = normp.tile([HD, S], F32, tag="yvar_t")
                    nc.vector.tensor_scalar(
                        out=yvar_t[:],
                        in0=t23b[:],
                        scalar1=l_var,
                        scalar2=colsum_sb[0:HD, h, :],
                        op0=ALU.mult,
                        op1=ALU.add,
                    )
                    nc.gpsimd.dma_start(
                        out=yvar_sb[off : off + HD, mt_h, :], in_=yvar_t[:]
                    )

            # ============== output projections + ReduceScatter ==============
            with (
                tc.tile_pool(name="tailp", bufs=2) as tailp,
                tc.tile_pool(name="ops", bufs=4, space="PSUM") as ops,
            ):
                bob_sb = tailp.tile([128, D], F32, tag="bob")
                nc.gpsimd.dma_start(out=bob_sb[:], in_=bob)

                def outproj(path, ysb, wsb):
                    for mt in range(NT):
                        row = 512 * (mt // 2) + 256 * path + 128 * (mt % 2)
                        pe = tailp.tile([128, D], F32, tag="pe")
                        for j in range(2):
                            po = ops.tile([128, 512], F32, tag="po")
                            for kt in range(2):
                                nc.tensor.matmul(
                                    po[:],
                                    lhsT=ysb[:, kt, 128 * mt : 128 * mt + 128],
                                    rhs=wsb[:, kt, 512 * j : 512 * j + 512],
                                    start=(kt == 0),
                                    stop=(kt == 1),
                                )
                            if path == 0:
                                nc.scalar.copy(pe[:, 512 * j : 512 * j + 512], po[:])
                            else:
                                nc.vector.tensor_copy(
                                    pe[:, 512 * j : 512 * j + 512], po[:]
                                )
                        nc.gpsimd.dma_start(
                            out=cc_in.ap()[row : row + 128, :], in_=pe[:]
                        )

                outproj(0, ymu_sb, wo_sb)
                for kt in range(2):
                    ysc = tailp.tile([128, S], F32, tag="ysc")
                    nc.scalar.activation(
                        ysc[:], yvar_sb[:, kt, :], AF.Sqrt, bias=EPS
                    )
                    nc.scalar.activation(
                        ysq_sb[:, kt, :], ysc[:], AF.Square, bias=EPS
                    )
                outproj(1, ysq_sb, wo2_sb)

                if TIMING_SINGLE:
                    nc.sync.dma_start(out=cc_out.ap(), in_=cc_in.ap()[0 : 2 * DL, :])
                else:
                    nc.gpsimd.collective_compute(
                        "ReduceScatter",
                        ALU.add,
                        replica_groups=[[0, 1, 2, 3], [4, 5, 6, 7]],
                        ins=[cc_in.ap()],
                        outs=[cc_out.ap()],
                    )
                for i in range(2):
                    r = slice(128 * i, 128 * i + 128)
                    lsb = tailp.tile([128, D], F32, tag="lsb")
                    nc.scalar.dma_start(out=lsb[:], in_=cc_out.ap()[r, :])
                    olt = tailp.tile([128, D], F32, tag="olt")
                    nc.vector.tensor_add(olt[:], lsb[:], bob_sb[:])
                    nc.sync.dma_start(out=out_loc[r, :], in_=olt[:])

                    vsb = tailp.tile([128, D], F32, tag="vsb")
                    nc.scalar.dma_start(
                        out=vsb[:], in_=cc_out.ap()[DL + 128 * i : DL + 128 * i + 128, :]
                    )
                    ost = tailp.tile([128, D], F32, tag="ost")
                    nc.scalar.activation(ost[:], vsb[:], AF.Sqrt)
                    nc.sync.dma_start(out=out_scale[r, :], in_=ost[:])

    nc.compile()
    return nc


def _prep_inputs(q_loc, v_loc, k_loc, v_scale, Wq, bq, Wk, bk, Wv, bv, Wo, bo):
    """Build the 8 per-core input dicts (host-side sharding/marshalling)."""
    f = np.float32
    per_batch = []
    for b in range(B):
        per_batch.append({
            "xq": np.ascontiguousarray(q_loc[b].T, dtype=f),
            "xk": np.ascontiguousarray(k_loc[b].T, dtype=f),
            "xv": np.ascontiguousarray(v_loc[b].T, dtype=f),
            "xvs2": np.ascontiguousarray(v_scale[b].T, dtype=f) ** 2,
        })

    cvec = np.full((128, 1), 1e-4 + EPS, dtype=f)
    onew = np.zeros((128, 2048), dtype=f)
    onew[:, 0::64] = 1.0
    bob = np.ascontiguousarray(np.broadcast_to(bo, (128, D)), dtype=f)

    per_group = []
    for g in range(G):
        dsl = slice(DL * g, DL * (g + 1))
        woT = np.ascontiguousarray(Wo[:, dsl].T, dtype=f)
        per_group.append({
            "wq": np.ascontiguousarray(Wq[dsl].T, dtype=f),
            "wk": np.ascontiguousarray(Wk[dsl].T, dtype=f),
            "wv": np.ascontiguousarray(Wv[dsl].T, dtype=f),
            "wo": woT,
            "bq": np.ascontiguousarray(bq[dsl, None], dtype=f),
            "bk": np.ascontiguousarray(bk[dsl, None], dtype=f),
            "bvb": np.ascontiguousarray(
                np.broadcast_to(bv[dsl], (128, DL)), dtype=f
            ),
        })

    in_maps = []
    for c in range(NCORES):
        b, g = c // G, c % G
        m = dict(per_batch[b])
        m.update(per_group[g])
        m.update({"bob": bob, "cvec": cvec, "onew": onew})
        in_maps.append(m)
    return in_maps


def kernel(q_loc, q_scale, k_loc, k_scale, v_loc, v_scale,
           Wq, bq, Wk, bk, Wv, bv, Wo, bo, tau):
    tau_f = float(np.asarray(tau))
    if tau_f not in _CACHE:
        _CACHE[tau_f] = _build(tau_f)
    nc = _CACHE[tau_f]

    in_maps = _prep_inputs(
        np.asarray(q_loc), np.asarray(v_loc), np.asarray(k_loc),
        np.asarray(v_scale), np.asarray(Wq), np.asarray(bq),
        np.asarray(Wk), np.asarray(bk), np.asarray(Wv), np.asarray(bv),
        np.asarray(Wo), np.asarray(bo),
    )
    res = run_bass_kernel_spmd(nc, in_maps, list(range(NCORES)))

    out_loc = np.empty((B, S, D), dtype=np.float32)
    out_scale = np.empty((B, S, D), dtype=np.float32)
    for c in range(NCORES):
        b, g = c // G, c % G
        rows = slice(DL * g, DL * (g + 1))
        out_loc[b, rows, :] = res.results[c]["out_loc"]
        out_scale[b, rows, :] = res.results[c]["out_scale"]
    return out_loc, out_scale
